# revision 34
# baseline (speedup 1.0000x reference)
"""Trainium2 Bass kernel for LocalSpatioTemporalPooling.

Reference computation (per sample n):
  x: (C=256, T=30, H=64, W=44) fp32
  feats[c,t,s] = mean over the (8,44) spatial stripe s of frame t    # 352-elem mean
  scores[t,s] = || feats[:,t,s] ||_2  (clip eps)                     # reduce over C
  top-2 frames per stripe by score; output[s*C + c] = mean of the 2 selected feats

Sharding: pure data parallel -- one sample per NeuronCore (N=8 = n_cores).

Kernel structure per core (x viewed as (C, T*S*352); 352-groups contiguous in HBM):
  - Stream chunks (c-block 0: 6x5 frames; c-block 1 tapers 5,4,4,4,3,3,2,2,1,1
    to t28, then frame 29 arrives as 4 stripe-pieces s0-3/s4-5/s6/s7) so each
    chunk's reduce fits inside the next chunk's DMA window and the post-stream
    tail ends with a single-stripe reduce.  Group sums are split between the
    DVE (tensor_reduce) and the otherwise-idle ACT engine (accumulate-copy per
    group).  Results land in feats (128, 480) = [c-block 0 | c-block 1],
    stripe-major (s*30 + t).
  - Scores accumulate DURING the stream: after each chunk's reduce, ACT squares
    that slice and PE accumulates ones^T @ sq into the (1, 240) PSUM sumsq.
    For the split frame 29, each stripe's c-block-1 contribution is a fused
    feats-column x feats-column matmul (skips the ACT square on the tail).
  - Top-2 runs INCREMENTALLY: (m1, m2) over t<=24 via reduce/mask/reduce in
    DVE idle windows mid-stream, then frames 25..27 fold in with 3-op (1,8)
    min/max updates as their scores complete.  After the last stripe's score
    lands only the t28+t29 folds + the (1,240) weight compare remain on the
    critical path.  (All fold/mask ops are on DVE: GPSIMD cannot access PSUM,
    and ACT has no two-tensor min/max.)
  - Weighted mean: PE broadcasts w to 128 partitions (2 bf16 matmuls); one
    DVE scalar_tensor_tensor prod = feats * WSCALE * wb; one fused strided
    DVE reduce -> oblk (128, 16) with (stripe, c-block)-interleaved columns
    so the PE transpose rows land in HBM row order; DVE copies the (16, 128)
    transpose to SBUF; one SP HWDGE DMA writes the output.
    (SWDGE prepare/trigger output paths were tried and are ~1.2us faster in
    the cost model, but the Q7 gather/scatter ucode mis-executes in this
    environment, so the plain HWDGE path stays.)
"""

import dataclasses

import numpy as np
from contextlib import ExitStack

import concourse.bass as bass
import concourse.tile as tile
import concourse.mybir as mybir
from concourse import bacc
from concourse.bass_utils import run_bass_kernel_spmd
from concourse.masks import make_identity

N, C, T, H, W = 8, 256, 30, 64, 44
S = 8                 # stripes
SH = H // S           # 8 rows per stripe
GROUP = SH * W        # 352 elements per (c, t, s) group
CB = C // 128         # 2 channel blocks
FRAME = H * W         # 2816
WSCALE = 0.5 / GROUP  # top-2 mean of stripe means
BIG = 1.0e30
TL = T - 1            # last frame index (29)
TRUN = 25             # running top-2 covers t < TRUN; t >= TRUN folds in

CHUNKS_CB0 = [(0, 5), (5, 5), (10, 5), (15, 5), (20, 5), (25, 5)]
CHUNKS_CB1 = [(0, 5), (5, 4), (9, 4), (13, 4), (17, 3), (20, 3), (23, 2),
              (25, 2), (27, 1), (28, 1)]                   # t29 via PIECES
# final frame (cb1, t=29) split by stripes: (s0, sn, engine)
PIECES = [(0, 4, "dve"), (4, 2, "act"), (6, 1, "dve"), (7, 1, "dve")]
# PE order for the per-stripe score stop-matmuls (earliest-ready first:
# DVE s0-3 -> ACT s4 -> DVE s6 -> ACT s5 -> DVE s7)
STOP_ORDER = [0, 1, 2, 3, 4, 6, 5, 7]

_F32 = mybir.dt.float32
_BF16 = mybir.dt.bfloat16
_I16 = mybir.dt.int16


def _bcast(ap2d, inner):
    """(1, K) AP -> (1, K, inner) stride-0 broadcast view."""
    [pp, pc], [fs, fc] = ap2d.ap[0], ap2d.ap[1]
    return dataclasses.replace(ap2d, ap=[[pp, pc], [fs, fc], [0, inner]])


def _col3(t):
    """(1, K) tile -> (1, K, 1) view (rank-matched for tensor_tensor)."""
    return t[:].rearrange("p (s o) -> p s o", o=1)


def _kernel_body(ctx, tc, nc, x, out, repeat=1):
    const_pool = ctx.enter_context(tc.tile_pool(name="const", bufs=1))
    in_pool = ctx.enter_context(tc.tile_pool(name="inp", bufs=3))
    piece_pool = ctx.enter_context(tc.tile_pool(name="piece", bufs=1))
    feat_pool = ctx.enter_context(tc.tile_pool(name="feat", bufs=1))
    small_pool = ctx.enter_context(tc.tile_pool(name="small", bufs=1))
    psum_pool = ctx.enter_context(tc.tile_pool(name="psum", bufs=1, space="PSUM"))

    ones_col = const_pool.tile([128, 1], _F32)
    nc.vector.memset(ones_col[:], 1.0)
    wrow = const_pool.tile([1, 128], _BF16)
    nc.vector.memset(wrow[:], 1.0)
    identity = const_pool.tile([128, 128], _F32)
    make_identity(nc, identity[:])
    # output staging for the final (16, 128) block
    outt = const_pool.tile([16, 128], _F32)
    out_rows = out.rearrange("s (b c) -> (s b) c", b=CB)     # row r = s*2+cb

    # [c-block 0 | c-block 1] side by side; free layout within a block: s*30 + t
    feats = feat_pool.tile([128, CB * T * S], _F32)
    sq = feat_pool.tile([128, CB * T * S], _F32)
    # t-major score layout (element t*S+s): each frame's 8 scores are one
    # contiguous 8-element interval, so Tile's interval-based dependency
    # tracking gives fold ops on frame t no false RAW edge against later
    # frames' matmuls (s-major layout serialized the t28 fold behind the
    # t29 stop-matmuls)
    ss_psum = psum_pool.tile([1, T * S], _F32, tag="ss")
    ssv_t = ss_psum[:].rearrange("p (t s) -> p t s", t=T)
    ssv = ss_psum[:].rearrange("p (t s) -> p s t", t=T)

    def fview(cb, t0, tc_, s0=0, sn=S):  # (128, tc_, sn) t-minor slice
        return feats[:, cb * T * S:(cb + 1) * T * S].rearrange(
            "p (s t) -> p t s", s=S)[:, t0:t0 + tc_, s0:s0 + sn]

    # scratch rows for ACT-side accumulate-copy (value discarded, accum kept)
    act_scratch = [feat_pool.tile([128, GROUP], _F32, name=f"actscr{i}")
                   for i in range(2)]
    act_n = [0]

    def act_group_sum(cb, src_ap, t_abs, s_):
        # sum one (c, t, s) 352-group on the Scalar engine via accum_out
        scr = act_scratch[act_n[0] % 2]
        act_n[0] += 1
        nc.scalar.activation(
            scr[:], src_ap,
            mybir.ActivationFunctionType.Copy,
            accum_out=feats[:, cb * T * S + s_ * T + t_abs:
                            cb * T * S + s_ * T + t_abs + 1],
        )

    def sview(t_, cb, t0, tc_):  # (128, 8, tc_) s-major slice
        return t_[:, cb * T * S:(cb + 1) * T * S].rearrange(
            "p (s t) -> p s t", s=S)[:, :, t0:t0 + tc_]

    def tview(t_, cb, t0, tc_):  # (128, tc_, 8) t-major strided slice
        return t_[:, cb * T * S:(cb + 1) * T * S].rearrange(
            "p (s t) -> p t s", s=S)[:, t0:t0 + tc_, :]

    for _rep in range(repeat):

        # running top-2 state; folds allocate fresh tiles per frame.
        # all fold/mask ops run on DVE: GPSIMD cannot access PSUM (BIR
        # verifier), and DVE has idle windows everywhere these land.
        m1 = [None]
        m2 = [None]
        mkd_ref = [None]

        def fold_frame(t_abs, last=False):
            # m2 <- max(m2, min(m1, ss_t)); m1 <- max(m1, ss_t)
            ss_t = ssv[:, :, t_abs:t_abs + 1]
            tmp = small_pool.tile([1, S], _F32, name=f"tmp{t_abs}")
            nc.vector.tensor_tensor(_col3(tmp), _col3(m1[0]), ss_t,
                                    op=mybir.AluOpType.min)
            m2n = small_pool.tile([1, S], _F32, name=f"m2f{t_abs}")
            nc.vector.tensor_tensor(_col3(m2n), _col3(m2[0]), _col3(tmp),
                                    op=mybir.AluOpType.max)
            m2[0] = m2n
            if not last:
                m1n = small_pool.tile([1, S], _F32, name=f"m1f{t_abs}")
                nc.vector.tensor_tensor(_col3(m1n), _col3(m1[0]), ss_t,
                                        op=mybir.AluOpType.max)
                m1[0] = m1n

        # ---- streamed reduction + in-stream score accumulation ----
        def process_chunk(cb, t0, tcn):
            tl = in_pool.tile([128, 5 * S * GROUP], _F32, name="tl", tag="tl")
            nc.sync.dma_start(
                tl[:, :tcn * S * GROUP],
                x[cb * 128:(cb + 1) * 128, t0 * FRAME:(t0 + tcn) * FRAME],
            )
            in4 = tl[:, :tcn * S * GROUP].rearrange("p (t s w) -> p t s w", t=tcn, s=S)
            # split the group-sums between DVE (tensor_reduce) and the
            # otherwise-idle ACT engine (accumulate-copy per group)
            dve_f, act_f = {5: (3, 2), 4: (2, 2), 3: (2, 1),
                            2: (1, 1), 1: (0, 0)}[tcn]
            if tcn == 1:
                # half a frame each: DVE takes stripes 0-3, ACT takes 4-7
                nc.vector.tensor_reduce(
                    fview(cb, t0, 1, 0, 4), in4[:, :, 0:4, :],
                    axis=mybir.AxisListType.X, op=mybir.AluOpType.add,
                )
                for s_ in range(4, S):
                    act_group_sum(cb, tl[:, s_ * GROUP:(s_ + 1) * GROUP], t0, s_)
            else:
                nc.vector.tensor_reduce(
                    fview(cb, t0, dve_f), in4[:, :dve_f, :, :],
                    axis=mybir.AxisListType.X, op=mybir.AluOpType.add,
                )
                for tloc in range(dve_f, tcn):
                    for s_ in range(S):
                        g = tloc * S + s_
                        act_group_sum(cb, tl[:, g * GROUP:(g + 1) * GROUP],
                                      t0 + tloc, s_)
            nc.scalar.activation(
                sview(sq, cb, t0, tcn), sview(feats, cb, t0, tcn),
                mybir.ActivationFunctionType.Square,
            )

        for (t0, tcn) in CHUNKS_CB0:
            process_chunk(0, t0, tcn)
        for (t0, tcn) in CHUNKS_CB1:
            process_chunk(1, t0, tcn)
            # both c-blocks' squares for this t-range are now available
            nc.tensor.matmul(
                ssv_t[:, t0:t0 + tcn, :], lhsT=ones_col[:],
                rhs=tview(sq, 0, t0, tcn), start=True, stop=False,
            )
            nc.tensor.matmul(
                ssv_t[:, t0:t0 + tcn, :], lhsT=ones_col[:],
                rhs=tview(sq, 1, t0, tcn), start=False, stop=True,
            )
            if t0 + tcn == TRUN:
                # ss[0:TRUN] complete -> base running top-2; the four ops fit
                # in the ~5us DVE idle window of this 2-frame chunk
                m1r = small_pool.tile([1, S], _F32, name="m1r")
                nc.vector.tensor_reduce(m1r[:], ssv[:, :, 0:TRUN],
                                        axis=mybir.AxisListType.X,
                                        op=mybir.AluOpType.max)
                eqr = small_pool.tile([1, S * TRUN], _F32, name="eqr")
                eqv = eqr[:].rearrange("p (s t) -> p s t", s=S)
                nc.vector.tensor_tensor(eqv, ssv[:, :, 0:TRUN],
                                        _bcast(m1r[:], TRUN),
                                        op=mybir.AluOpType.is_ge)
                mkd = small_pool.tile([1, S * TRUN], _F32, name="mkd")
                mkv = mkd[:].rearrange("p (s t) -> p s t", s=S)
                nc.vector.scalar_tensor_tensor(
                    mkv, eqv, -BIG, ssv[:, :, 0:TRUN],
                    op0=mybir.AluOpType.mult, op1=mybir.AluOpType.add,
                )
                m2r = small_pool.tile([1, S], _F32, name="m2r")
                nc.vector.tensor_reduce(m2r[:], mkv,
                                        axis=mybir.AxisListType.X,
                                        op=mybir.AluOpType.max)
                mkd_ref[0] = mkd
                m1[0] = m1r
                m2[0] = m2r
            elif t0 >= TRUN and t0 + tcn < T:
                for ta in range(t0, t0 + tcn):
                    if ta < T - 2:  # t28 folds after the piece reduces
                        fold_frame(ta)

        # ---- final frame (cb1, t29) streamed as 4 stripe-pieces ----
        ptiles = {}
        for (s0, sn, eng) in PIECES:
            pt = piece_pool.tile([128, sn * GROUP], _F32, name=f"p{s0}")
            ptiles[s0] = pt
            nc.sync.dma_start(
                pt[:],
                x[128:256, TL * FRAME + s0 * GROUP:
                  TL * FRAME + (s0 + sn) * GROUP],
            )
        for (s0, sn, eng) in PIECES:
            pt = ptiles[s0]
            if eng == "dve":
                nc.vector.tensor_reduce(
                    fview(1, TL, 1, s0, sn),
                    pt[:].rearrange("p (t s w) -> p t s w", t=1, s=sn),
                    axis=mybir.AxisListType.X, op=mybir.AluOpType.add,
                )
            else:
                for s_ in range(s0, s0 + sn):
                    act_group_sum(1, pt[:, (s_ - s0) * GROUP:(s_ - s0 + 1) * GROUP],
                                  TL, s_)
        # fold t28 BEFORE emitting the t29 score matmuls: Tile orders PSUM
        # reads after every previously-emitted accumulation group on the
        # tile closes, so emitting the fold first lets it run as soon as the
        # t28 matmul lands instead of trailing the t29 stop-matmuls
        fold_frame(T - 2)
        # frame-29 scores: c-block 0 via ones^T @ sq (inputs ready long ago,
        # runs as soon as PE drains the chunk matmuls -- off critical path),
        # then per-stripe c-block-1 contributions as fused feats^2 column
        # matmuls (each fires as its stripe's group sum lands)
        nc.tensor.matmul(
            ssv_t[:, TL:TL + 1, :], lhsT=ones_col[:],
            rhs=tview(sq, 0, TL, 1), start=True, stop=False,
            skip_group_check=True,
        )
        for s_ in STOP_ORDER:
            fcol = feats[:, T * S + s_ * T + TL:T * S + s_ * T + TL + 1]
            nc.tensor.matmul(
                ss_psum[:, TL * S + s_:TL * S + s_ + 1],
                lhsT=fcol, rhs=fcol, start=False, stop=True,
                skip_group_check=True,
            )
        ss29 = ssv[:, :, TL:TL + 1]
        tmp29 = small_pool.tile([1, S], _F32, name="tmp29")
        nc.vector.tensor_tensor(_col3(tmp29), _col3(m1[0]), ss29,
                                op=mybir.AluOpType.min)
        m2fin = small_pool.tile([1, S], _F32, name="m2fin")
        nc.vector.tensor_tensor(_col3(m2fin), _col3(m2[0]), _col3(tmp29),
                                op=mybir.AluOpType.max)
        w = small_pool.tile([1, T * S], _BF16, name="w")
        wv = w[:].rearrange("p (s t) -> p s t", s=S)
        nc.vector.tensor_tensor(wv, ssv, _bcast(m2fin[:], T),
                                op=mybir.AluOpType.is_ge)

        # ---- weighted frame mean (bf16 prod: 2x DVE throughput; wb is an
        # exact 0/1 mask and the output tolerance is 2e-2) ----
        wb_psum = psum_pool.tile([128, CB * T * S], _F32, tag="wb")
        for cb in range(CB):
            nc.tensor.matmul(wb_psum[:, cb * T * S:(cb + 1) * T * S],
                             lhsT=wrow[:], rhs=w[:], start=True, stop=True)
        prod = small_pool.tile([128, CB * T * S], _F32, name="prod")
        nc.vector.scalar_tensor_tensor(
            prod[:], feats[:], WSCALE, wb_psum[:],
            op0=mybir.AluOpType.mult, op1=mybir.AluOpType.mult,
        )
        # oblk columns interleaved (s, cb) so the transposed rows match the
        # HBM row order
        oblk = small_pool.tile([128, CB * S], _F32, name="oblk")
        ov = oblk[:].rearrange("p (s b) -> p b s", b=CB)
        nc.vector.tensor_reduce(
            ov, prod[:].rearrange("p (b s t) -> p b s t", b=CB, s=S),
            axis=mybir.AxisListType.X, op=mybir.AluOpType.add,
        )
        tr_psum = psum_pool.tile([CB * S, 128], _F32, tag="tr")
        nc.tensor.transpose(tr_psum[:], oblk[:], identity[:])
        nc.vector.tensor_copy(outt[:], tr_psum[:])
        nc.sync.dma_start(out_rows, outt[:])


_NC_CACHE = {}


def _get_nc(repeat=1):
    if repeat not in _NC_CACHE:
        nc = bacc.Bacc("TRN2", target_bir_lowering=False, debug=False)
        x = nc.dram_tensor("x", [C, T * FRAME], _F32, kind="ExternalInput").ap()
        out = nc.dram_tensor("out", [S, C], _F32, kind="ExternalOutput").ap()
        with tile.TileContext(nc) as tc:
            with ExitStack() as ctx:
                _kernel_body(ctx, tc, nc, x, out, repeat=repeat)
        nc.compile()
        _NC_CACHE[repeat] = nc
    return _NC_CACHE[repeat]


def kernel(x):
    x = np.asarray(x, dtype=np.float32)
    assert x.shape == (N, C, T, H, W), x.shape
    nc = _get_nc()
    in_maps = [{"x": np.ascontiguousarray(x[i]).reshape(C, T * FRAME)} for i in range(N)]
    res = run_bass_kernel_spmd(nc, in_maps, list(range(N)))
    return np.stack([res.results[i]["out"].reshape(S * C) for i in range(N)])
